# revision 9
# baseline (speedup 1.0000x reference)
"""Conv2d 3x3 (stride 1, pad 1) NCHW kernel for 8 Trainium2 NeuronCores.

Problem: x (32,128,56,56) f32, weight (256,128,3,3), bias (256,)
         -> out (32,256,56,56), same-padding conv + bias.

Strategy (v3 — fp8 DoubleRow implicit GEMM):
  - Data parallel: 4 images per core across 8 cores (batch shard).
  - Implicit GEMM like the f32r baseline (input channels on the SBUF
    partition dim, one matmul per 3x3 tap against a shifted spatial
    window, 9 taps accumulated in PSUM), but the matmuls run in fp8e4
    DoubleRow perf mode: each instruction contracts TWO 128-deep
    k-tiles at 0.5 PE cycles per output column — 4x the FLOP rate of
    f32r/bf16.
  - Precision recovery (e4m3 alone is ~3.5% rel err, gate is 2e-2):
      x is split on the host into x_hi = e4m3(x) and
      x_lo = e4m3(x - x_hi); every tap's matmul pairs k-tiles
      [x_hi; x_lo] against duplicated weights [w_q; w_q], making the
      x-side exact to ~0.1%.  The w-side error is cancelled by 4
      correction matmuls pairing adjacent taps [x_hi(t); x_hi(t+1)]
      against [w_lo(t); w_lo(t+1)] where w_lo = e4m3(w - w_q); taps
      0-7 are corrected, tap 8's w-error (~0.8% residual) is left.
      Measured end-to-end rel err ~8e-3 with the bf16 output store.
  - Per (image, 8-row chunk, oc-half) group: 9 main + 4 correction
    DoubleRow matmuls (13 x 224 PE cycles vs the baseline's 9 x 448).
    Bias is fused into the PSUM->SBUF eviction (ACT/DVE alternating),
    outputs stored as bf16 (halves store traffic) and widened on the
    host.
  - The correction matmuls address the two x_hi tap-windows of a pair
    via a hand-built access pattern whose leading free dim strides
    between the two tap offsets inside the padded [58,58] plane.
"""

import numpy as np
import ml_dtypes

N_CORES = 8
N, C, H, W = 32, 128, 56, 56
O = 256
KH = KW = 3
PAD = 1
HP, WP = H + 2 * PAD, W + 2 * PAD  # 58, 58
NPC = N // N_CORES  # images per core = 4
RPC = 8  # output rows per chunk
N_CHUNKS = H // RPC  # 7
OC_TILES = O // 128  # 2
NPAIR = 4  # corrected tap pairs: (0,1) (2,3) (4,5) (6,7)

_CACHE = {}
LAST_RESULTS = None


def _build():
    import concourse.bass as bass
    import concourse.bacc as bacc
    import concourse.mybir as mybir
    import concourse.tile as tile
    from concourse.ap import AP

    f32 = mybir.dt.float32
    bf16 = mybir.dt.bfloat16
    f8 = mybir.dt.float8e4
    DR = mybir.MatmulPerfMode.DoubleRow

    nc = bacc.Bacc(
        "TRN2", target_bir_lowering=False, debug=False, num_devices=N_CORES
    )
    x_d = nc.dram_tensor("x8", (NPC, C, KW, 2, HP, W), f8, kind="ExternalInput")
    wm_d = nc.dram_tensor("wm", (C, KH * KW, 2, O), f8, kind="ExternalInput")
    wc_d = nc.dram_tensor("wc", (C, NPAIR, 2, O), f8, kind="ExternalInput")
    b_d = nc.dram_tensor("b2", (128, OC_TILES), f32, kind="ExternalInput")
    out_d = nc.dram_tensor("out", (NPC, O, H, W), bf16, kind="ExternalOutput")

    # Corrected tap pairs, ordered so the intra-pair offset delta in the
    # [KW, 2, HP, W] free space is positive: off(kh,kw) = (kw*2*HP + kh)*W.
    PAIRS = [(0, 1), (3, 2), (4, 5), (6, 7)]

    def pair_rhs(x_t, y0, p):
        # Correction rhs: k-tile i is the x_hi window of tap PAIRS[p][i],
        # addressed via an extra [delta, 2] free dim on the AP.
        t0, t1 = PAIRS[p]
        kh0, kw0 = divmod(t0, KW)
        kh1, kw1 = divmod(t1, KW)
        delta = ((kw1 - kw0) * 2 * HP + (kh1 - kh0)) * W
        assert delta > 0
        base = x_t[:, kw0, 0, y0 + kh0 : y0 + kh0 + RPC, :]
        ap = [list(d) for d in base.ap]
        return AP(
            tensor=base.tensor,
            offset=base.offset,
            ap=[ap[0], [delta, 2]] + ap[1:],
        )

    with tile.TileContext(nc) as tc:
        with (
            tc.tile_pool(name="w", bufs=1) as wpool,
            tc.tile_pool(name="x", bufs=2) as xpool,
            tc.tile_pool(name="ps", bufs=4, space=bass.MemorySpace.PSUM) as pspool,
            tc.tile_pool(name="o", bufs=6) as opool,
        ):
            wm_t = wpool.tile([C, KH * KW, 2, O], f8)
            wc_t = wpool.tile([C, NPAIR, 2, O], f8)
            b_t = wpool.tile([128, OC_TILES], f32)
            # PE pre-warm: the p-state model needs ~3us of continuous PE
            # busy to reach 2.4 GHz.  Run dummy matmuls on a scratch tile
            # while the startup DMAs land so the real matmuls start at
            # (or near) full clock.
            warm_t = wpool.tile([128, 2, 128], f8)
            warm_ps = pspool.tile([128, 64], f32, name="warm_ps", bufs=1)
            nc.vector.memset(warm_t[:], 0.0)
            for _ in range(130):
                nc.tensor.matmul(
                    warm_ps[:],
                    warm_t[:],
                    warm_t[:, :, 0:64],
                    start=True,
                    stop=True,
                    perf_mode=DR,
                )
            # Startup: oc=0 main weights, then the first chunks' input
            # rows, then the rest in deadline order.
            nc.scalar.dma_start(wm_t[:, :, :, 0:128], wm_d[:, :, :, 0:128])

            x_tiles = {}
            for n in range(NPC):
                if n == 0:
                    x_t = xpool.tile([C, KW, 2, HP, W], f8, name="x_t")
                    x_tiles[0] = x_t
                    head = RPC + 2  # rows for chunk 0
                    nc.scalar.dma_start(
                        x_t[:, :, :, 0:head, :], x_d[n, :, :, :, 0:head, :]
                    )
                    nc.scalar.dma_start(
                        wc_t[:, :, :, 0:128], wc_d[:, :, :, 0:128]
                    )
                    nc.scalar.dma_start(b_t[:], b_d[:])
                    nc.scalar.dma_start(
                        x_t[:, :, :, head:HP, :], x_d[n, :, :, :, head:HP, :]
                    )
                    nc.scalar.dma_start(
                        wm_t[:, :, :, 128:256], wm_d[:, :, :, 128:256]
                    )
                    nc.scalar.dma_start(
                        wc_t[:, :, :, 128:256], wc_d[:, :, :, 128:256]
                    )
                else:
                    x_t = x_tiles[n]
                if n + 1 < NPC:
                    # Prefetch the next image ahead of this image's
                    # evictions in ACT-ring program order.
                    x_next = xpool.tile([C, KW, 2, HP, W], f8, name="x_t")
                    x_tiles[n + 1] = x_next
                    nc.scalar.dma_start(x_next[:], x_d[n + 1])
                # Image 0 runs oc-major so the first groups only need the
                # oc=0 weight halves (the oc=1 halves land later).
                if n == 0:
                    group_iter = [
                        (ch, oc)
                        for oc in range(OC_TILES)
                        for ch in range(N_CHUNKS)
                    ]
                else:
                    group_iter = [
                        (ch, oc)
                        for ch in range(N_CHUNKS)
                        for oc in range(OC_TILES)
                    ]
                for ch, oc in group_iter:
                    y0 = ch * RPC
                    if True:
                        ocs = slice(oc * 128, (oc + 1) * 128)
                        ps = pspool.tile([128, RPC, W], f32)
                        k = 0
                        for kh in range(KH):
                            for kw in range(KW):
                                nc.tensor.matmul(
                                    ps[:],
                                    wm_t[:, kh * KW + kw, :, ocs],
                                    x_t[:, kw, :, y0 + kh : y0 + kh + RPC, :],
                                    start=(k == 0),
                                    stop=False,
                                    perf_mode=DR,
                                )
                                k += 1
                        for p in range(NPAIR):
                            nc.tensor.matmul(
                                ps[:],
                                wc_t[:, p, :, ocs],
                                pair_rhs(x_t, y0, p),
                                start=False,
                                stop=(p == NPAIR - 1),
                                perf_mode=DR,
                            )
                        o_t = opool.tile([128, RPC, W], bf16)
                        bias_ap = b_t[:, oc : oc + 1]
                        out_ap = out_d[n, ocs, y0 : y0 + RPC, :]
                        is_last = (
                            n == NPC - 1
                            and ch == N_CHUNKS - 1
                            and oc == OC_TILES - 1
                        )
                        if is_last:
                            # final group: halve the eviction across
                            # ACT+DVE and the store across both HWDGE
                            # rings to shorten the kernel tail.
                            hh = RPC // 2
                            nc.scalar.add(
                                o_t[:, 0:hh, :], ps[:, 0:hh, :], bias_ap
                            )
                            nc.vector.tensor_scalar_add(
                                o_t[:, hh:RPC, :], ps[:, hh:RPC, :], bias_ap
                            )
                            nc.sync.dma_start(
                                out_ap[:, 0:hh, :], o_t[:, 0:hh, :]
                            )
                            nc.scalar.dma_start(
                                out_ap[:, hh:RPC, :], o_t[:, hh:RPC, :]
                            )
                        elif (ch * OC_TILES + oc) % 2 == 0:
                            nc.scalar.add(o_t[:], ps[:], bias_ap)
                            nc.sync.dma_start(out_ap, o_t[:])
                        else:
                            nc.vector.tensor_scalar_add(o_t[:], ps[:], bias_ap)
                            nc.sync.dma_start(out_ap, o_t[:])
    nc.compile()
    return nc


def kernel(x, weight, bias):
    global LAST_RESULTS
    from concourse.bass_utils import run_bass_kernel_spmd

    x = np.asarray(x, dtype=np.float32)
    weight = np.asarray(weight, dtype=np.float32)
    bias = np.asarray(bias, dtype=np.float32)

    e4m3 = ml_dtypes.float8_e4m3

    xp = np.zeros((N, C, HP, WP), np.float32)
    xp[:, :, PAD : PAD + H, PAD : PAD + W] = x
    x_hi = xp.astype(e4m3)
    x_lo = (xp - x_hi.astype(np.float32)).astype(e4m3)
    # (N, C, KW, 2, HP, W): per-kw pre-shifted windows, hi/lo planes
    x8 = np.stack(
        [
            np.stack([x_hi[:, :, :, kw : kw + W], x_lo[:, :, :, kw : kw + W]], axis=2)
            for kw in range(KW)
        ],
        axis=2,
    )
    x8 = np.ascontiguousarray(x8)

    w_q = weight.astype(e4m3)
    w_lo = (weight - w_q.astype(np.float32)).astype(e4m3)
    # wm[c, t, i, o] = w_q[o, c, kh, kw] for i in {0, 1}
    wq_t = np.ascontiguousarray(
        w_q.astype(np.float32).transpose(1, 2, 3, 0)
    ).reshape(C, KH * KW, O)
    wm = np.repeat(wq_t[:, :, None, :], 2, axis=2).astype(e4m3)
    # wc[c, p, i, o] = w_lo[o, c, tap PAIRS[p][i]]
    wl_t = np.ascontiguousarray(
        w_lo.astype(np.float32).transpose(1, 2, 3, 0)
    ).reshape(C, KH * KW, O)
    PAIRS = [(0, 1), (3, 2), (4, 5), (6, 7)]
    wc = np.stack(
        [np.stack([wl_t[:, a], wl_t[:, b]], axis=1) for a, b in PAIRS], axis=1
    ).astype(e4m3)
    b2 = np.ascontiguousarray(bias.reshape(OC_TILES, 128).T)

    if "nc" not in _CACHE:
        _CACHE["nc"] = _build()
    nc = _CACHE["nc"]

    in_maps = [
        {"x8": x8[i * NPC : (i + 1) * NPC], "wm": wm, "wc": wc, "b2": b2}
        for i in range(N_CORES)
    ]
    res = run_bass_kernel_spmd(nc, in_maps, core_ids=list(range(N_CORES)))
    LAST_RESULTS = res
    out = np.concatenate([r["out"] for r in res.results], axis=0)
    return out.astype(np.float32)
